# revision 1
# baseline (speedup 1.0000x reference)
"""Trainium2 Bass kernel for nn_IsocortexSubstrate.

The reference network is three chained single-step SSM layers, each applied to
a fresh (all-zero) hidden state.  With h_prev = 0 the recurrent term
h_prev @ A.T vanishes, so layer k reduces to

    y_k = x_k * dot(B_k, C_k)          (per element)
    spikes_k = (sigmoid(y_k) > 0.5) = (y_k > 0)

Since spikes are in {0, 1}, chaining three layers collapses to a single
elementwise op on the input:

    out = ( x * m > 0 )   where   m = s1 * [s2 > 0] * [s3 > 0],
                                  s_k = dot(B_k, C_k)

This is a pure streaming elementwise kernel: read 16 MiB, write 16 MiB.
Sharding: pure data parallel over the batch dim across 8 cores; the six
16-element B/C vectors are replicated and the three dot products are
recomputed on every core (they cost nothing).
"""

import sys

sys.path.insert(0, "/opt/trn_rl_repo")

import numpy as np

N_CORES = 8
BATCH = 4096
WIDTH = 1024
ROWS = BATCH // N_CORES          # 512 rows per core
P = 128                          # SBUF partitions
COLS = ROWS * WIDTH // P         # 4096 f32 per partition per core
TILE_W = 512                     # column tile width
N_TILES = COLS // TILE_W

_cache = {}


def _build():
    import concourse.bacc as bacc
    import concourse.bass as bass
    import concourse.mybir as mybir
    from concourse.tile import TileContext

    f32 = mybir.dt.float32
    mult = mybir.AluOpType.mult
    is_gt = mybir.AluOpType.is_gt

    nc = bacc.Bacc(
        "TRN2",
        target_bir_lowering=False,
        debug=False,
        enable_asserts=False,
        num_devices=N_CORES,
    )
    x_in = nc.dram_tensor("x", [P, COLS], f32, kind="ExternalInput")
    bc_in = nc.dram_tensor("bc", [6, 16], f32, kind="ExternalInput")
    y_out = nc.dram_tensor("y", [P, COLS], f32, kind="ExternalOutput")

    with TileContext(nc) as tc:
        with (
            tc.tile_pool(name="singles", bufs=1) as singles,
            tc.tile_pool(name="io", bufs=4) as io,
        ):
            # Load the six 16-vectors broadcast to all 128 partitions.
            bc_t = singles.tile([P, 6, 16], f32)
            bc_ap = bc_in.ap()
            bc_bcast = bass.AP(
                tensor=bc_ap.tensor, offset=bc_ap.offset, ap=[[0, P], *bc_ap.ap]
            )
            nc.gpsimd.dma_start(out=bc_t[:], in_=bc_bcast)

            # s[:, k] = dot(B_k, C_k), per partition.
            prod = singles.tile([P, 3, 16], f32)
            nc.vector.tensor_mul(prod[:], bc_t[:, 0:3, :], bc_t[:, 3:6, :])
            s = singles.tile([P, 3, 1], f32)
            nc.vector.reduce_sum(s[:], prod[:], axis=mybir.AxisListType.X)

            # m = s1 * [s2 > 0] * [s3 > 0]
            g2 = singles.tile([P, 1], f32)
            nc.vector.tensor_scalar(
                out=g2[:], in0=s[:, 1, :], scalar1=0.0, scalar2=None, op0=is_gt
            )
            g3 = singles.tile([P, 1], f32)
            nc.vector.tensor_scalar(
                out=g3[:], in0=s[:, 2, :], scalar1=0.0, scalar2=None, op0=is_gt
            )
            m12 = singles.tile([P, 1], f32)
            nc.vector.tensor_mul(m12[:], s[:, 0, :], g2[:])
            m = singles.tile([P, 1], f32)
            nc.vector.tensor_mul(m[:], m12[:], g3[:])

            # Stream x through: out = (x * m) > 0
            xa = x_in.ap()
            ya = y_out.ap()
            for j in range(N_TILES):
                sl = slice(j * TILE_W, (j + 1) * TILE_W)
                xt = io.tile([P, TILE_W], f32)
                nc.sync.dma_start(out=xt[:], in_=xa[:, sl])
                ot = io.tile([P, TILE_W], f32)
                nc.vector.tensor_scalar(
                    out=ot[:],
                    in0=xt[:],
                    scalar1=m[:],
                    scalar2=0.0,
                    op0=mult,
                    op1=is_gt,
                )
                nc.sync.dma_start(out=ya[:, sl], in_=ot[:])

    nc.compile()
    return nc


def _get_nc():
    if "nc" not in _cache:
        _cache["nc"] = _build()
    return _cache["nc"]


def kernel(
    incoming_spikes,
    A_sensory, B_sensory, C_sensory,
    A_association, B_association, C_association,
    A_executive, B_executive, C_executive,
):
    from concourse.bass_utils import run_bass_kernel_spmd

    nc = _get_nc()

    x = np.ascontiguousarray(np.asarray(incoming_spikes, dtype=np.float32))
    bc = np.stack(
        [
            np.asarray(B_sensory, dtype=np.float32).reshape(16),
            np.asarray(B_association, dtype=np.float32).reshape(16),
            np.asarray(B_executive, dtype=np.float32).reshape(16),
            np.asarray(C_sensory, dtype=np.float32).reshape(16),
            np.asarray(C_association, dtype=np.float32).reshape(16),
            np.asarray(C_executive, dtype=np.float32).reshape(16),
        ]
    )

    shards = x.reshape(N_CORES, P, COLS)
    in_maps = [{"x": shards[i], "bc": bc} for i in range(N_CORES)]
    res = run_bass_kernel_spmd(nc, in_maps, list(range(N_CORES)))
    out = np.concatenate(
        [res.results[i]["y"].reshape(ROWS, WIDTH) for i in range(N_CORES)], axis=0
    )
    return out


# revision 11
# speedup vs baseline: 1.0287x; 1.0287x over previous
"""Trainium2 Bass kernel for nn_IsocortexSubstrate.

The reference network is three chained single-step SSM layers, each applied to
a fresh (all-zero) hidden state.  With h_prev = 0 the recurrent term
h_prev @ A.T vanishes, so layer k reduces to

    y_k = x_k * dot(B_k, C_k)          (per element)
    spikes_k = (sigmoid(y_k) > 0.5) = (y_k > 0)

Since spikes are in {0, 1}, chaining three layers collapses to a single
elementwise op on the input:

    out = ( x * m > 0 )   where   m = s1 * [s2 > 0] * [s3 > 0],
                                  s_k = dot(B_k, C_k)

This is a pure streaming elementwise kernel: read 16 MiB, write 16 MiB.
Sharding: pure data parallel over the batch dim across 8 cores; the six
16-element B/C vectors are replicated (broadcast-loaded into all 128 SBUF
partitions) and the three dot products are recomputed per partition on every
core (they cost nothing).

Implementation is raw Bass (no Tile framework): the Tile preamble/tail
barriers and per-op semaphore machinery cost ~9 us on a ~12 us-roofline
kernel.  Manual schedule:

  sync engine:   HWDGE broadcast-load of the B/C vectors, then 4 x 512 KiB
                 input-chunk loads (contiguous DRAM, 128-partition tiles)
  vector engine: 6 tiny ops to compute m per partition, then one fused
                 tensor_scalar (mult, is_gt) per chunk
  scalar engine: 4 x 512 KiB output stores on the second HWDGE ring,
                 final wait for store completion
"""

import sys

sys.path.insert(0, "/opt/trn_rl_repo")

import numpy as np

N_CORES = 8
BATCH = 4096
WIDTH = 1024
ROWS = BATCH // N_CORES          # 512 rows per core
P = 128                          # SBUF partitions
COLS = ROWS * WIDTH // P         # 4096 f32 per partition per core
N_CHUNKS = 4
CHUNK = COLS // N_CHUNKS         # 1024 f32; one chunk = contiguous 512 KiB

_cache = {}


def _build():
    import concourse.bass as bass
    import concourse.mybir as mybir

    f32 = mybir.dt.float32
    mult = mybir.AluOpType.mult
    is_gt = mybir.AluOpType.is_gt

    nc = bass.Bass("TRN2", target_bir_lowering=False, debug=False,
                   enable_asserts=False, num_devices=N_CORES)
    x_in = nc.dram_tensor("x", [N_CHUNKS, P, CHUNK], f32, kind="ExternalInput")
    bc_in = nc.dram_tensor("bc", [6, 16], f32, kind="ExternalInput")
    y_out = nc.dram_tensor("y", [N_CHUNKS, P, CHUNK], f32, kind="ExternalOutput")

    import contextlib

    with contextlib.ExitStack() as stack:
        bc_sem = stack.enter_context(nc.semaphore("bc_in"))
        x_sems = [
            stack.enter_context(nc.semaphore(f"x_in{c}")) for c in range(N_CHUNKS)
        ]
        v_done = stack.enter_context(nc.semaphore("v_done"))
        dma_out = stack.enter_context(nc.semaphore("dma_out"))
        sb = stack.enter_context
        bc_t = sb(nc.sbuf_tensor("bc_t", [P, 6, 16], f32))
        prod = sb(nc.sbuf_tensor("prod", [P, 3, 16], f32))
        s = sb(nc.sbuf_tensor("s", [P, 3, 1], f32))
        g2 = sb(nc.sbuf_tensor("g2", [P, 1], f32))
        g3 = sb(nc.sbuf_tensor("g3", [P, 1], f32))
        m12 = sb(nc.sbuf_tensor("m12", [P, 1], f32))
        m = sb(nc.sbuf_tensor("m", [P, 1], f32))
        xt = sb(nc.sbuf_tensor("xt", [P, COLS], f32))
        ot = sb(nc.sbuf_tensor("ot", [P, COLS], f32))

        xa = x_in.ap()
        ya = y_out.ap()
        bca = bc_in.ap()
        # Source AP reading the same 6x16 block into every partition.
        bc_bcast = bass.AP(tensor=bca.tensor, offset=bca.offset,
                           ap=[[0, P], *bca.ap])

        with nc.Block() as block:

            @block.sync
            def _(sync):
                sync.dma_start(out=bc_t[:], in_=bc_bcast).then_inc(bc_sem, 16)
                for c in range(N_CHUNKS):
                    sync.dma_start(
                        out=xt[:, c * CHUNK:(c + 1) * CHUNK], in_=xa[c]
                    ).then_inc(x_sems[c], 16)

            @block.vector
            def _(vector):
                vector.wait_ge(bc_sem, 16)
                vector.tensor_mul(prod[:], bc_t[:, 0:3, :], bc_t[:, 3:6, :])
                vector.drain()
                vector.reduce_sum(s[:], prod[:], axis=mybir.AxisListType.X)
                vector.drain()
                vector.tensor_scalar(
                    out=g2[:], in0=s[:, 1, :], scalar1=0.0, scalar2=None, op0=is_gt
                )
                vector.tensor_scalar(
                    out=g3[:], in0=s[:, 2, :], scalar1=0.0, scalar2=None, op0=is_gt
                )
                vector.drain()
                vector.tensor_mul(m12[:], s[:, 0, :], g2[:])
                vector.drain()
                vector.tensor_mul(m[:], m12[:], g3[:])
                vector.drain()
                for c in range(N_CHUNKS):
                    vector.wait_ge(x_sems[c], 16)
                    cs = slice(c * CHUNK, (c + 1) * CHUNK)
                    vector.tensor_scalar(
                        out=ot[:, cs], in0=xt[:, cs],
                        scalar1=m[:], scalar2=0.0, op0=mult, op1=is_gt,
                    ).then_inc(v_done, 1)

            @block.scalar
            def _(scalar):
                for c in range(N_CHUNKS):
                    scalar.wait_ge(v_done, c + 1)
                    scalar.dma_start(
                        out=ya[c], in_=ot[:, c * CHUNK:(c + 1) * CHUNK]
                    ).then_inc(dma_out, 16)
                scalar.wait_ge(dma_out, 16 * N_CHUNKS)

    return nc


def _get_nc():
    if "nc" not in _cache:
        _cache["nc"] = _build()
    return _cache["nc"]


def kernel(
    incoming_spikes,
    A_sensory, B_sensory, C_sensory,
    A_association, B_association, C_association,
    A_executive, B_executive, C_executive,
):
    from concourse.bass_utils import run_bass_kernel_spmd

    nc = _get_nc()

    x = np.ascontiguousarray(np.asarray(incoming_spikes, dtype=np.float32))
    bc = np.stack(
        [
            np.asarray(B_sensory, dtype=np.float32).reshape(16),
            np.asarray(B_association, dtype=np.float32).reshape(16),
            np.asarray(B_executive, dtype=np.float32).reshape(16),
            np.asarray(C_sensory, dtype=np.float32).reshape(16),
            np.asarray(C_association, dtype=np.float32).reshape(16),
            np.asarray(C_executive, dtype=np.float32).reshape(16),
        ]
    )

    shards = x.reshape(N_CORES, N_CHUNKS, P, CHUNK)
    in_maps = [{"x": shards[i], "bc": bc} for i in range(N_CORES)]
    res = run_bass_kernel_spmd(nc, in_maps, list(range(N_CORES)))
    out = np.concatenate(
        [res.results[i]["y"].reshape(ROWS, WIDTH) for i in range(N_CORES)], axis=0
    )
    return out


# revision 13
# speedup vs baseline: 1.0348x; 1.0059x over previous
"""Trainium2 Bass kernel for nn_IsocortexSubstrate.

The reference network is three chained single-step SSM layers, each applied to
a fresh (all-zero) hidden state.  With h_prev = 0 the recurrent term
h_prev @ A.T vanishes, so layer k reduces to

    y_k = x_k * dot(B_k, C_k)          (per element)
    spikes_k = (sigmoid(y_k) > 0.5) = (y_k > 0)

Since spikes are in {0, 1}, chaining three layers collapses to a single
elementwise op on the input:

    out = ( x * m > 0 )   where   m = s1 * [s2 > 0] * [s3 > 0],
                                  s_k = dot(B_k, C_k)

This is a pure streaming elementwise kernel: read 16 MiB, write 16 MiB.
Sharding: pure data parallel over the batch dim across 8 cores.

Implementation is raw Bass (no Tile framework; the Tile preamble/tail
barriers cost several us on a ~12 us-roofline kernel).  Schedule:

  sync engine:   one 384 B load of the B/C vectors (transposed, into 16
                 partitions), then input chunks 0-1, then output stores 2-3
  gpsimd engine: input chunks 2-3 via SWDGE (parallel issue ring)
  vector engine: [16,3] elementwise B*C products; after the PE broadcast,
                 the m chain; then one fused tensor_scalar (mult, is_gt)
                 per 512 KiB chunk
  tensor engine: broadcast-reduce ones[16,128].T @ prodT[16,3] -> PSUM
                 [128,3], giving every partition all three dot products
  scalar engine: output stores 0-1 on the second HWDGE ring

DMA completion semaphores are per-transfer (completion order across
transfers is not guaranteed); same-engine RAW hazards on the DVE pipeline
are covered by DRAIN.
"""

import sys

sys.path.insert(0, "/opt/trn_rl_repo")

import numpy as np

N_CORES = 8
BATCH = 4096
WIDTH = 1024
ROWS = BATCH // N_CORES          # 512 rows per core
P = 128                          # SBUF partitions
COLS = ROWS * WIDTH // P         # 4096 f32 per partition per core
N_CHUNKS = 4
CHUNK = COLS // N_CHUNKS         # 1024 f32; one chunk = contiguous 512 KiB

_cache = {}


def _build():
    import contextlib

    import concourse.bass as bass
    import concourse.mybir as mybir

    f32 = mybir.dt.float32
    mult = mybir.AluOpType.mult
    is_gt = mybir.AluOpType.is_gt

    nc = bass.Bass("TRN2", target_bir_lowering=False, debug=False,
                   enable_asserts=False, num_devices=N_CORES)
    x_in = nc.dram_tensor("x", [N_CHUNKS, P, CHUNK], f32, kind="ExternalInput")
    bc_in = nc.dram_tensor("bc", [6, 16], f32, kind="ExternalInput")
    y_out = nc.dram_tensor("y", [N_CHUNKS, P, CHUNK], f32, kind="ExternalOutput")

    with contextlib.ExitStack() as stack:
        sem = lambda name: stack.enter_context(nc.semaphore(name))
        bc_sem = sem("bc_in")
        x_sems = [sem(f"x_in{c}") for c in range(N_CHUNKS)]
        v_pre = sem("v_pre")
        mm_sem = sem("mm")
        v_done = sem("v_done")
        so_sync = sem("so_sync")
        so_scal = sem("so_scal")

        sb = stack.enter_context
        bcT = sb(nc.sbuf_tensor("bcT", [16, 6], f32))
        prodT = sb(nc.sbuf_tensor("prodT", [16, 3], f32))
        ones = sb(nc.sbuf_tensor("ones", [16, 128], f32))
        sall = sb(nc.psum_tensor("sall", [128, 3], f32))
        g2 = sb(nc.sbuf_tensor("g2", [P, 1], f32))
        g3 = sb(nc.sbuf_tensor("g3", [P, 1], f32))
        m12 = sb(nc.sbuf_tensor("m12", [P, 1], f32))
        m = sb(nc.sbuf_tensor("m", [P, 1], f32))
        xt = sb(nc.sbuf_tensor("xt", [P, COLS], f32))
        ot = sb(nc.sbuf_tensor("ot", [P, COLS], f32))

        xa = x_in.ap()
        ya = y_out.ap()
        bca = bc_in.ap()
        # bc is [6,16] row-major in DRAM; read it transposed into [16,6]:
        # partition stride 1 (along the 16-dim), free stride 16 (across the
        # six vectors).
        bcT_src = bass.AP(tensor=bca.tensor, offset=bca.offset,
                          ap=[[1, 16], [16, 6]])

        with nc.Block() as block:

            @block.sync
            def _(sync):
                with nc.allow_non_contiguous_dma(
                    reason="96 x 4B transposed load of the tiny B/C block"
                ):
                    sync.dma_start(out=bcT[:], in_=bcT_src).then_inc(bc_sem, 16)
                for c in (0, 1):
                    sync.dma_start(
                        out=xt[:, c * CHUNK:(c + 1) * CHUNK], in_=xa[c]
                    ).then_inc(x_sems[c], 16)
                for c in (2, 3):
                    sync.wait_ge(v_done, c + 1)
                    sync.dma_start(
                        out=ya[c], in_=ot[:, c * CHUNK:(c + 1) * CHUNK]
                    ).then_inc(so_sync, 16)
                sync.wait_ge(so_sync, 32)

            @block.gpsimd
            def _(gpsimd):
                for c in (2, 3):
                    gpsimd.dma_start(
                        out=xt[:, c * CHUNK:(c + 1) * CHUNK], in_=xa[c]
                    ).then_inc(x_sems[c], 16)

            @block.tensor
            def _(tensor):
                tensor.wait_ge(v_pre, 1)
                tensor.matmul(sall[:], ones[:], prodT[:]).then_inc(mm_sem, 1)

            @block.vector
            def _(vector):
                vector.memset(ones[:], 1.0)
                vector.wait_ge(bc_sem, 16)
                vector.tensor_mul(prodT[:], bcT[:, 0:3], bcT[:, 3:6])
                vector.drain()
                vector.sem_inc(v_pre, 1)
                vector.wait_ge(mm_sem, 1)
                vector.tensor_scalar(
                    out=g2[:], in0=sall[:, 1:2], scalar1=0.0, scalar2=None,
                    op0=is_gt,
                )
                vector.tensor_scalar(
                    out=g3[:], in0=sall[:, 2:3], scalar1=0.0, scalar2=None,
                    op0=is_gt,
                )
                vector.drain()
                vector.tensor_mul(m12[:], sall[:, 0:1], g2[:])
                vector.drain()
                vector.tensor_mul(m[:], m12[:], g3[:])
                vector.drain()
                for c in range(N_CHUNKS):
                    vector.wait_ge(x_sems[c], 16)
                    cs = slice(c * CHUNK, (c + 1) * CHUNK)
                    vector.tensor_scalar(
                        out=ot[:, cs], in0=xt[:, cs],
                        scalar1=m[:], scalar2=0.0, op0=mult, op1=is_gt,
                    ).then_inc(v_done, 1)

            @block.scalar
            def _(scalar):
                for c in (0, 1):
                    scalar.wait_ge(v_done, c + 1)
                    scalar.dma_start(
                        out=ya[c], in_=ot[:, c * CHUNK:(c + 1) * CHUNK]
                    ).then_inc(so_scal, 16)
                scalar.wait_ge(so_scal, 32)

    return nc


def _get_nc():
    if "nc" not in _cache:
        _cache["nc"] = _build()
    return _cache["nc"]


def kernel(
    incoming_spikes,
    A_sensory, B_sensory, C_sensory,
    A_association, B_association, C_association,
    A_executive, B_executive, C_executive,
):
    from concourse.bass_utils import run_bass_kernel_spmd

    nc = _get_nc()

    x = np.ascontiguousarray(np.asarray(incoming_spikes, dtype=np.float32))
    bc = np.stack(
        [
            np.asarray(B_sensory, dtype=np.float32).reshape(16),
            np.asarray(B_association, dtype=np.float32).reshape(16),
            np.asarray(B_executive, dtype=np.float32).reshape(16),
            np.asarray(C_sensory, dtype=np.float32).reshape(16),
            np.asarray(C_association, dtype=np.float32).reshape(16),
            np.asarray(C_executive, dtype=np.float32).reshape(16),
        ]
    )

    shards = x.reshape(N_CORES, N_CHUNKS, P, CHUNK)
    in_maps = [{"x": shards[i], "bc": bc} for i in range(N_CORES)]
    res = run_bass_kernel_spmd(nc, in_maps, list(range(N_CORES)))
    out = np.concatenate(
        [res.results[i]["y"].reshape(ROWS, WIDTH) for i in range(N_CORES)], axis=0
    )
    return out
